# revision 2
# baseline (speedup 1.0000x reference)
"""Trainium2 Bass kernel v3: baseline schedule + fp8-split projections.

Identical attention scheduling to the proven v1 kernel (fp32r S/O',
transposed-layout softmax, interleaved emission). Changes:
- QKV and V projections run as 3-term split-fp8 DoubleRow matmuls
  (x_hi*W_hi + x_hi*W_lo + x_lo*W_hi); W_qkv is scaled x16 on the host
  so its hi/lo e4m3 split resolves (values ~0.036 underflow e4m3's
  subnormal range otherwise), and the gate vector w is pre-divided by
  16 to cancel the scale. Hi/lo fp8 pairs are host-side re-encodings.
- Batched DMA loads (one DMA per tensor; HWDGE costs ~630ns each).
- Output projection writes f16 (halves the output DMA) with junk
  matmuls padding the last norm's latency so the tail projection runs
  at full PE clock.
"""

from contextlib import ExitStack

import numpy as np
import ml_dtypes

import concourse.bass as bass
import concourse.mybir as mybir
import concourse.tile as tile
from concourse import bacc
from concourse.bass import ts
from concourse.bass_utils import run_bass_kernel_spmd

B, N, D, H = 8, 1024, 768, 12
HD = D // H          # 64
SCALE = HD ** -0.5   # 0.125
KC = D // 128        # 6
KC2 = D // 256       # 3 double-row contraction chunks
MC_QK = (2 * D) // 128  # 12 row-chunks of [q;k]^T
NT = N // 128        # 8 token chunks
NC2 = N // 512       # 2 moving chunks

F32 = mybir.dt.float32
F32R = mybir.dt.float32r
F16 = mybir.dt.float16
F8 = mybir.dt.float8e4
AF = mybir.ActivationFunctionType
DR = mybir.MatmulPerfMode.DoubleRow
E4 = ml_dtypes.float8_e4m3

_CACHE: dict = {}


def _run(gen):
    for _ in gen:
        pass


def _emit(tc, repeat=1):
    nc = tc.nc
    x8h_d = nc.dram_tensor("x8h", [KC2, 2, 128, N], F8, kind="ExternalInput").ap()
    x8l_d = nc.dram_tensor("x8l", [KC2, 2, 128, N], F8, kind="ExternalInput").ap()
    w_d = nc.dram_tensor("w", [1, N], F32, kind="ExternalInput").ap()
    wqh_d = nc.dram_tensor("wqh", [KC2, 2, 128, 2 * D], F8, kind="ExternalInput").ap()
    wql_d = nc.dram_tensor("wql", [KC2, 2, 128, 2 * D], F8, kind="ExternalInput").ap()
    wvh_d = nc.dram_tensor("wvh", [KC2, 2, 128, D], F8, kind="ExternalInput").ap()
    wvl_d = nc.dram_tensor("wvl", [KC2, 2, 128, D], F8, kind="ExternalInput").ap()
    wm_d = nc.dram_tensor("wmsa", [D, D], F32R, kind="ExternalInput").ap()
    bm_d = nc.dram_tensor("bmsa", [D], F32, kind="ExternalInput").ap()
    y_d = nc.dram_tensor("yt", [D, N], F16, kind="ExternalOutput").ap()

    for _rep in range(repeat):
        _emit_body(tc, x8h_d, x8l_d, w_d, wqh_d, wql_d, wvh_d, wvl_d, wm_d, bm_d, y_d)


def _emit_body(tc, x8h_d, x8l_d, w_d, wqh_d, wql_d, wvh_d, wvl_d, wm_d, bm_d, y_d):
    nc = tc.nc
    with ExitStack() as s1:
        const = s1.enter_context(tc.tile_pool(name="const", bufs=1))
        pwm = s1.enter_context(tc.tile_pool(name="pwm", bufs=1))
        pwv = s1.enter_context(tc.tile_pool(name="pwv", bufs=1))
        pqk = s1.enter_context(tc.tile_pool(name="pqk", bufs=1))
        pv = s1.enter_context(tc.tile_pool(name="pv", bufs=1))
        pot = s1.enter_context(tc.tile_pool(name="pot", bufs=1))
        px8 = s1.enter_context(tc.tile_pool(name="px8", bufs=1))
        pwqs = s1.enter_context(tc.tile_pool(name="pwqs", bufs=2))
        pe_ = s1.enter_context(tc.tile_pool(name="pe", bufs=5))
        pdn = s1.enter_context(tc.tile_pool(name="pdn", bufs=2))
        pbc = s1.enter_context(tc.tile_pool(name="pbc", bufs=1))
        pfin = s1.enter_context(tc.tile_pool(name="pfin", bufs=2))
        psA = s1.enter_context(tc.tile_pool(name="psA", bufs=2, space="PSUM"))
        psB = s1.enter_context(tc.tile_pool(name="psB", bufs=2, space="PSUM"))

        # ---- startup-critical loads ----
        x8h = px8.tile([128, KC2 * 2 * N], F8, tag="x8h")
        x8l = px8.tile([128, KC2 * 2 * N], F8, tag="x8l")
        x8h4 = x8h[:].rearrange("p (c s n) -> p c s n", s=2, n=N)
        x8l4 = x8l[:].rearrange("p (c s n) -> p c s n", s=2, n=N)
        w_row = pdn.tile([1, N], F32, tag="dn", name="w_row")
        nc.scalar.dma_start(w_row[:], w_d[:])
        nc.sync.dma_start(x8h4[:], x8h_d.rearrange("c s p n -> p c s n"))
        nc.scalar.dma_start(x8l4[:], x8l_d.rearrange("c s p n -> p c s n"))

        def load_wq8(m, eng=None):
            eng = eng or nc.sync
            th = pwqs.tile([128, KC2 * 2 * 128], F8, tag="wqh", name=f"wqh{m}")
            tl = pwqs.tile([128, KC2 * 2 * 128], F8, tag="wql", name=f"wql{m}")
            th4 = th[:].rearrange("p (c s e) -> p c s e", s=2, e=128)
            tl4 = tl[:].rearrange("p (c s e) -> p c s e", s=2, e=128)
            eng.dma_start(th4, wqh_d[:, :, :, ts(m, 128)].rearrange("c s p e -> p c s e"))
            eng.dma_start(tl4, wql_d[:, :, :, ts(m, 128)].rearrange("c s p e -> p c s e"))
            return th, th4, tl4

        wq_pre = load_wq8(0, nc.sync)
        wq_pre2 = load_wq8(KC, nc.scalar)

        onescol_f = const.tile([128, HD], F32, tag="onescol_f")
        nc.vector.memset(onescol_f[:], 1.0)
        onescol = const.tile([128, HD], F32R, tag="onescol")
        nc.vector.tensor_copy(onescol[:], onescol_f[:])

        # ---- PE warm-up: junk matmuls during the DMA wait ----
        psj = psA.tile([128, 512], F32, tag="psA", name="psj")
        for _ in range(12):
            nc.tensor.matmul(
                psj[0:HD, 0:HD], onescol[:], onescol[:], start=True, stop=True
            )
        for _ in range(6):
            nc.tensor.matmul(
                psj[:], x8h[:, 0:128], x8h[:, 0:512], start=True, stop=True
            )
        for _ in range(6):
            nc.tensor.matmul(
                psj[:], wq_pre[0][:, 0:128], x8h[:, 0:512], start=True, stop=True
            )

        # non-critical loads
        wv8h = pwv.tile([128, KC2 * 2 * D], F8, tag="wv8h")
        wv8l = pwv.tile([128, KC2 * 2 * D], F8, tag="wv8l")
        wv8h4 = wv8h[:].rearrange("p (c s e) -> p c s e", s=2, e=D)
        wv8l4 = wv8l[:].rearrange("p (c s e) -> p c s e", s=2, e=D)
        nc.scalar.dma_start(wv8h4, wvh_d.rearrange("c s p e -> p c s e"))
        nc.scalar.dma_start(wv8l4, wvl_d.rearrange("c s p e -> p c s e"))
        bias = const.tile([128, KC], F32, tag="bias")
        nc.sync.dma_start(bias[:], bm_d.rearrange("(c p) -> p c", p=128))

        wb = const.tile([128, N], F32, tag="wb")
        nc.gpsimd.partition_broadcast(wb[:], w_row[:])
        wcol = const.tile([128, NT], F32, tag="wcol")
        nc.sync.dma_start(wcol[:], w_d[0, :].rearrange("(r p) -> p r", p=128))

        qkt = [pqk.tile([128, N], F32R, tag=f"qk{m}", name=f"qk{m}") for m in range(MC_QK)]
        vt = [
            pv.tile([128, H * (HD + 1)], F32R, tag=f"v{r}", name=f"v{r}")
            for r in range(NT)
        ]
        ott = [pot.tile([128, N], F32R, tag=f"ot{c}", name=f"ot{c}") for c in range(KC)]
        wmt = [pwm.tile([128, D], F32R, tag=f"wm{c}", name=f"wm{c}") for c in range(KC)]

        def gen_qk(m, pre=None):
            """qk^T chunk m via 3-term split fp8. Yields per (term, c2)."""
            _, th4, tl4 = pre if pre is not None else load_wq8(m)
            ps = psB.tile([128, N], F32, tag="psB", name="psB")
            i = 0
            for xa, wa in ((x8h4, th4), (x8l4, th4), (x8h4, tl4)):
                for c2 in range(KC2):
                    for j in range(NC2):
                        nc.tensor.matmul(
                            ps[:, ts(j, 512)],
                            wa[:, c2],
                            xa[:, c2, :, ts(j, 512)],
                            start=(i == 0),
                            stop=(i == 3 * KC2 - 1),
                            perf_mode=DR,
                        )
                    i += 1
                    yield
            # deferred copy (baseline): gate applied on PSUM->SBUF
            yield
            nc.vector.tensor_mul(qkt[m][:], ps[:], wb[:])

        def gen_v():
            """V in natural layout + ones column, 3-term split fp8."""
            for r in range(NT):
                pvp = psB.tile([128, D], F32, tag="psB", name="psB")
                i = 0
                for xa, wa in ((x8h4, wv8h4), (x8l4, wv8h4), (x8h4, wv8l4)):
                    for c2 in range(KC2):
                        for off, wd in ((0, 512), (512, 256)):
                            nc.tensor.matmul(
                                pvp[:, off : off + wd],
                                xa[:, c2, :, ts(r, 128)],
                                wa[:, c2, :, off : off + wd],
                                start=(i == 0),
                                stop=(i == 3 * KC2 - 1),
                                perf_mode=DR,
                            )
                        i += 1
                v3 = vt[r][:].rearrange("p (h e) -> p h e", e=HD + 1)
                nc.vector.tensor_copy(
                    v3[:, :, HD : HD + 1],
                    onescol[:, 0:H].rearrange("p (h o) -> p h o", o=1),
                )
                nc.vector.tensor_scalar_mul(
                    v3[:, :, 0:HD],
                    pvp[:].rearrange("p (h e) -> p h e", e=HD),
                    wcol[:, r : r + 1],
                )
                yield

        def do_o(h, r, e, po):
            for j in range(NC2):
                nc.tensor.matmul(
                    po[:, ts(j, 512)],
                    vt[r][:, h * (HD + 1) : (h + 1) * (HD + 1)],
                    e[:, ts(j, 512)],
                    start=(r == 0),
                    stop=(r == NT - 1),
                )

        def norm_head(h, po):
            dnr = pdn.tile([1, N], F32, tag="dn", name="dnr")
            nc.vector.tensor_copy(dnr[:], po[HD : HD + 1, :])
            dn = pdn.tile([1, N], F32, tag="dn", name="dn")
            nc.vector.reciprocal_approx_fast(dn[:], dnr[:])
            bc = pbc.tile([HD, N], F32, tag="bc", name="bc")
            orow = ott[h // 2][HD * (h % 2) : HD * (h % 2) + HD, :]
            for j in range(NC2):
                nc.gpsimd.partition_broadcast(bc[:, ts(j, 512)], dn[:, ts(j, 512)])
                nc.vector.tensor_mul(
                    orow[:, ts(j, 512)],
                    po[0:HD, ts(j, 512)],
                    bc[:, ts(j, 512)],
                )

        def gen_attn(h):
            """Attention head h; S pipelined 2 ahead of O' (baseline)."""
            qt, qr = qkt[h // 2], HD * (h % 2)
            kt, kr = qkt[KC + h // 2], HD * (h % 2)
            po = psB.tile([HD + 1, N], F32, tag="psB", name="psB")
            pend = []
            for r in range(NT):
                ps = psA.tile([128, N], F32, tag="psA", name="psA")
                for j in range(NC2):
                    nc.tensor.matmul(
                        ps[:, ts(j, 512)],
                        kt[kr : kr + HD, ts(r, 128)],
                        qt[qr : qr + HD, ts(j, 512)],
                        start=True,
                        stop=True,
                    )
                e = pe_.tile([128, N], F32R, tag="e", name="e")
                nc.scalar.activation(e[:], ps[:], AF.Exp, scale=SCALE)
                if len(pend) == 4:
                    do_o(h, *pend.pop(0), po)
                pend.append((r, e))
                yield
            for r_, e_ in pend:
                do_o(h, r_, e_, po)
            norm_head(h, po)

        def interleave(main, filler, skip=0, ratio=1.5):
            owed = 0.0
            for i, _ in enumerate(main):
                if i >= skip:
                    owed += ratio
                    while owed >= 1.0:
                        next(filler, None)
                        owed -= 1.0
            _run(filler)

        def chain(*gens):
            for g in gens:
                yield from g

        # ---- schedule (baseline structure) ----
        _run(gen_qk(0, pre=wq_pre))
        _run(gen_qk(KC, pre=wq_pre2))

        def gen_proj(c, ps, k_from, k_to):
            for k in range(k_from, k_to):
                for j in range(NC2):
                    nc.tensor.matmul(
                        ps[:, ts(j, 512)],
                        wmt[k][:, ts(c, 128)],
                        ott[k][:, ts(j, 512)],
                        start=(k == 0),
                        stop=(k == KC - 1),
                    )
                yield

        def finish_proj(c, ps, k_from):
            _run(gen_proj(c, ps, k_from, KC))
            fin = pfin.tile([128, N], F16, tag="fin", name="fin")
            nc.scalar.activation(fin[:], ps[:], AF.Identity, bias=bias[:, c : c + 1])
            eng = nc.sync if c % 2 == 0 else nc.scalar
            eng.dma_start(y_d[ts(c, 128), :], fin[:])

        interleave(gen_v(), gen_attn(0), ratio=1.0)
        filler_map = {
            1: [1, KC + 1, 2],
            2: [KC + 2],
            3: [3],
            4: [KC + 3],
            5: [4],
            6: [KC + 4],
            7: [5],
            8: [KC + 5],
        }
        for h in range(1, H):
            chunks = filler_map.get(h, [])
            if chunks:
                interleave(
                    gen_attn(h),
                    chain(*[gen_qk(m) for m in chunks]),
                    skip=2,
                    ratio=len(chunks) * 10.0 / (NT - 2),
                )
            else:
                _run(gen_attn(h))
            if h == 2:
                for c in range(KC):
                    nc.sync.dma_start(wmt[c][:], wm_d[ts(c, 128), :])

        # ---- output projection + bias ----
        # junk matmuls keep the PE pstate hot through the final norm chain
        psjt = psA.tile([128, 512], F32, tag="psA", name="psjt")
        for _ in range(16):
            nc.tensor.matmul(
                psjt[0:HD, 0:HD], onescol[:], onescol[:], start=True, stop=True
            )
        for c in range(KC):
            ps = psA.tile([128, N], F32, tag="psA", name="psA")
            finish_proj(c, ps, 0)


def _build(repeat=1):
    key = ("nc", repeat)
    if key not in _CACHE:
        nc = bacc.Bacc("TRN2", target_bir_lowering=False, debug=False, num_devices=B)
        with tile.TileContext(nc) as tc:
            _emit(tc, repeat=repeat)
        nc.compile()
        _CACHE[key] = nc
    return _CACHE[key]


def _split8(a):
    """[768, X] f32 -> (hi, lo) each [KC2, 2, 128, X] e4m3 pair layout."""
    hi = a.astype(E4)
    lo = (a - hi.astype(np.float32)).astype(E4)
    x = a.shape[1]
    return (
        np.ascontiguousarray(hi.reshape(KC2, 2, 128, x)),
        np.ascontiguousarray(lo.reshape(KC2, 2, 128, x)),
    )


def kernel(x, weight, W_qkv, W_msa, b_msa):
    nc = _build()
    x = np.asarray(x, dtype=np.float32)
    weight = np.asarray(weight, dtype=np.float32)
    W_qkv = np.asarray(W_qkv, dtype=np.float32)
    wqh, wql = _split8(np.ascontiguousarray(W_qkv[:, : 2 * D]) * 16.0)
    wvh, wvl = _split8(np.ascontiguousarray(W_qkv[:, 2 * D :]) * 16.0)
    wm = np.asarray(W_msa, dtype=np.float32)
    bm = np.asarray(b_msa, dtype=np.float32)
    in_maps = []
    for b in range(B):
        x8h, x8l = _split8(np.ascontiguousarray(x[b].T))
        in_maps.append(
            {
                "x8h": x8h,
                "x8l": x8l,
                "w": np.ascontiguousarray(weight[b : b + 1] / 16.0),
                "wqh": wqh,
                "wql": wql,
                "wvh": wvh,
                "wvl": wvl,
                "wmsa": wm,
                "bmsa": bm,
            }
        )
    res = run_bass_kernel_spmd(nc, in_maps, list(range(B)))
    out = np.stack(
        [res.results[b]["yt"].astype(np.float32).T for b in range(B)], axis=0
    )
    return np.ascontiguousarray(out)
